# revision 22
# baseline (speedup 1.0000x reference)
"""Trainium2 Bass kernel for nn_LocalInteraction (SpookyNet-style local interaction).

Strategy (8 NeuronCores, SPMD):
  - Edges sharded by DESTINATION node: core c owns nodes [2000c, 2000c+2000)
    plus all edges whose receiver i lies there; 16 windows of 128 dest nodes,
    padded to a uniform per-window tile grid shared by all cores (one NEFF).
  - Radial prologue (batched): d/u/Y2/bernstein basis for ALL edges in a few
    large DVE/ACT ops; bern transposed once globally to bernT (lhsT layout).
  - Node phase (replicated, bf16): three edge MLPs over all 16384 nodes; the
    third layer is computed as swT @ W3 per 128-node block so the output is
    node-major directly (no PE transposes); records [node, 384] bf16 in HBM.
  - Edge phase: records gathered by j via dma_gather on rotating SWDGE queues
    (parallel Q7 pairs); radG = bernT @ G via PE; A = rec * radG read straight
    from PSUM by DVE; scatter-add = 4 PSUM-accumulated one-hot matmuls/tile.
  - Window epilogue: P/D invariant contractions + final MLP -> output [f, n];
    host reassembles and transposes.
"""
import sys, os, math
if not any("trn_rl_repo" in p or "simrepo" in p for p in sys.path):
    sys.path.insert(0, "/opt/trn_rl_repo")
import numpy as np

import concourse.bass as bass
import concourse.bacc as bacc
import concourse.mybir as mybir
import concourse.tile as tile
from concourse.bass_utils import run_bass_kernel_spmd
from concourse.masks import make_identity
from concourse.tile import add_dep_helper

F32 = mybir.dt.float32
BF16 = mybir.dt.bfloat16
I16 = mybir.dt.int16
AF = mybir.ActivationFunctionType
ALU = mybir.AluOpType

N_NODES = 16000
FEAT = 128
GAMMA = 0.5
R_CUT = 5.0
N_CORES = 8
NPC = N_NODES // N_CORES          # 2000 nodes per core
NW = (NPC + 127) // 128           # 16 windows per core
NPAD = 16384
NCHUNK = NPAD // 512              # 32
LOCPAD = NW * 128                 # 2048
_BINOM = np.array([math.comb(15, k) for k in range(16)], np.float64)

LAST_EXEC_NS = None
LAST_RESULT = None
_prog_cache = {}


def _build_program(n_t, share_sw=False, b3zero=False):
    nt_list = [x for x in n_t if not isinstance(x, str)]
    TT = sum(nt_list)
    starts = np.concatenate([[0], np.cumsum(nt_list)]).astype(int)
    ntmax = max(nt_list)
    NG = sum((nt + 3) // 4 for nt in nt_list)   # total 4-tile groups

    nc = bacc.Bacc("TRN2", target_bir_lowering=False, debug=False,
                   num_devices=N_CORES, num_swdge_queues=4)
    for v in (1e-12,):
        t_ = nc.alloc_sbuf_tensor(f"const-float32-{v}", [128, 1], F32)
        nc.gpsimd.memset(t_.ap(), v)
        nc.const_aps.aps[(F32, v)] = t_.ap()
    nc.all_engine_barrier()

    # fp32 consts: svec(30) gk(16) k15(16) lnb(16)
    CF32 = 30 + 48
    # bf16 consts: iota(128) erel(TT) wsb(1920) grep(384) pdm(512)
    #              xtl(2048) b3rep(384)
    CB16 = 128 + TT + 1920 + 384 + 512 + 2048 + 384
    cstd = nc.dram_tensor("cstd", [128, CF32], F32, kind="ExternalInput")
    cstb = nc.dram_tensor("cstb", [128, CB16], BF16, kind="ExternalInput")
    xtT = nc.dram_tensor("xtT", [128, NPAD], BF16, kind="ExternalInput")
    eidx = nc.dram_tensor("eidx", [128, TT * 8], I16, kind="ExternalInput")
    erij = nc.dram_tensor("erij", [128, TT * 3], F32, kind="ExternalInput")
    outT = nc.dram_tensor("outT", [128, LOCPAD], F32, kind="ExternalOutput")
    mrec = nc.dram_tensor("mrec", [NPAD, 384], BF16)

    def sv(idx, col):
        c = idx * 6 + col
        return svec_sb[:, c:c + 1]

    def wslice(idx, layer):
        k = 3 * idx + layer
        return wsb[:, k * 128:(k + 1) * 128]

    with tile.TileContext(nc) as tc:
        from contextlib import ExitStack
        es = ExitStack()
        cst = es.enter_context(tc.tile_pool(name="cst", bufs=1))

        cst_sb = cst.tile([128, CF32], F32)
        nc.sync.dma_start(out=cst_sb[:], in_=cstd[:])
        svec_sb = cst_sb[:, 0:30]
        gk_b = cst_sb[:, 30:46].rearrange("p (o k) -> p o k", o=1)
        k15_b = cst_sb[:, 46:62].rearrange("p (o k) -> p o k", o=1)
        lnb_b = cst_sb[:, 62:78].rearrange("p (o k) -> p o k", o=1)

        cstb_sb = cst.tile([128, CB16], BF16)
        nc.sync.dma_start(out=cstb_sb[:], in_=cstb[:])
        o = 0
        iotae = cstb_sb[:, o:o + 128]; o += 128
        erele = cstb_sb[:, o:o + TT]; o += TT
        wsb = cstb_sb[:, o:o + 1920]; o += 1920
        grep_sb = cstb_sb[:, o:o + 384]; o += 384
        pdm_sb = cstb_sb[:, o:o + 512]; o += 512
        xtl_sb = cstb_sb[:, o:o + 2048]; o += 2048
        b3rep = cstb_sb[:, o:o + 384]; o += 384

        eidx_sb = cst.tile([128, TT * 8], I16)
        nc.sync.dma_start(out=eidx_sb[:], in_=eidx[:])
        erij_sb = cst.tile([128, TT, 3], F32)
        nc.sync.dma_start(out=erij_sb[:],
                          in_=erij[:].rearrange("p (t c) -> p t c", c=3))
        xt_sb = cst.tile([128, NPAD], BF16)
        nc.sync.dma_start(out=xt_sb[:], in_=xtT[:])

        ident = cst.tile([128, 128], F32)
        make_identity(nc, ident[:])
        identb = cst.tile([128, 128], BF16)
        nc.vector.tensor_copy(identb[:], ident[:])

        # persistent radial outputs
        ub = cst.tile([128, TT, 3], BF16)        # unit vector (bf16)
        y2b = cst.tile([128, TT, 5], BF16)       # Y2 spherical (bf16)
        bernT = cst.tile([128, NG, 128], BF16)   # transposed bern (lhsT layout)
        cterm = cst.tile([128, LOCPAD], F32)

        # ------------------------------------------------------------------
        # radial prologue: one batched chain over all TT tiles
        # ------------------------------------------------------------------
        with (
            tc.tile_pool(name="rad", bufs=1) as radp,
            tc.tile_pool(name="rps", bufs=2, space="PSUM") as rps,
        ):
            rijv = erij_sb[:]
            sq = radp.tile([128, TT, 3], F32)
            nc.vector.tensor_tensor(out=sq[:], in0=rijv, in1=rijv, op=ALU.mult)
            d2 = radp.tile([128, TT, 1], F32)
            nc.vector.tensor_reduce(out=d2[:], in_=sq[:],
                                    axis=mybir.AxisListType.X, op=ALU.add)
            d = radp.tile([128, TT, 1], F32)
            nc.scalar.activation(d[:], d2[:], AF.Sqrt, bias=1e-12)
            inv_d = radp.tile([128, TT, 1], F32)
            nc.vector.reciprocal(inv_d[:], d[:])
            u = radp.tile([128, TT, 3], F32)
            nc.vector.tensor_tensor(out=u[:], in0=rijv,
                                    in1=inv_d[:].to_broadcast([128, TT, 3]),
                                    op=ALU.mult)
            nc.vector.tensor_copy(ub[:], u[:])
            usq = radp.tile([128, TT, 3], F32)
            nc.vector.tensor_tensor(out=usq[:], in0=u[:], in1=u[:], op=ALU.mult)
            y2 = radp.tile([128, TT, 5], F32)
            nc.vector.tensor_tensor(out=y2[:, :, 0:1], in0=u[:, :, 0:1],
                                    in1=u[:, :, 1:2], op=ALU.mult)
            nc.vector.tensor_tensor(out=y2[:, :, 1:2], in0=u[:, :, 0:1],
                                    in1=u[:, :, 2:3], op=ALU.mult)
            nc.vector.tensor_tensor(out=y2[:, :, 2:3], in0=u[:, :, 1:2],
                                    in1=u[:, :, 2:3], op=ALU.mult)
            nc.vector.tensor_tensor(out=y2[:, :, 3:4], in0=usq[:, :, 0:1],
                                    in1=usq[:, :, 1:2], op=ALU.subtract)
            nc.vector.tensor_scalar(out=y2[:, :, 4:5], in0=usq[:, :, 2:3],
                                    scalar1=3.0, scalar2=-1.0,
                                    op0=ALU.mult, op1=ALU.add)
            nc.vector.tensor_copy(y2b[:], y2[:])

            rho = radp.tile([128, TT, 1], F32)
            nc.scalar.activation(rho[:], d[:], AF.Exp, scale=-GAMMA)
            om = radp.tile([128, TT, 1], F32)
            nc.vector.tensor_scalar(out=om[:], in0=rho[:], scalar1=-1.0,
                                    scalar2=1.0, op0=ALU.mult, op1=ALU.add)
            nc.vector.tensor_scalar(out=om[:], in0=om[:], scalar1=1e-38,
                                    scalar2=None, op0=ALU.max)
            lg = radp.tile([128, TT, 1], F32)
            nc.scalar.activation(lg[:], om[:], AF.Ln)
            den = radp.tile([128, TT, 1], F32)
            nc.vector.tensor_scalar(out=den[:], in0=d2[:], scalar1=-1.0,
                                    scalar2=R_CUT * R_CUT,
                                    op0=ALU.mult, op1=ALU.add)
            rden = radp.tile([128, TT, 1], F32)
            nc.vector.reciprocal(rden[:], den[:])
            mme = radp.tile([128, TT, 1], F32)
            nc.vector.tensor_tensor(out=mme[:], in0=d2[:], in1=rden[:],
                                    op=ALU.mult)
            msk = radp.tile([128, TT, 1], F32)
            nc.vector.tensor_scalar(out=msk[:], in0=d[:], scalar1=R_CUT,
                                    scalar2=None, op0=ALU.is_ge)
            arge = radp.tile([128, TT, 1], F32)
            nc.vector.scalar_tensor_tensor(out=arge[:], in0=msk[:], scalar=1e30,
                                           in1=mme[:], op0=ALU.mult, op1=ALU.add)
            argv = radp.tile([128, TT, 16], F32)
            nc.vector.tensor_tensor(out=argv[:],
                                    in0=d[:].to_broadcast([128, TT, 16]),
                                    in1=gk_b.to_broadcast([128, TT, 16]),
                                    op=ALU.mult)
            t16 = radp.tile([128, TT, 16], F32)
            nc.vector.tensor_tensor(out=t16[:],
                                    in0=lg[:].to_broadcast([128, TT, 16]),
                                    in1=k15_b.to_broadcast([128, TT, 16]),
                                    op=ALU.mult)
            nc.vector.tensor_tensor(out=argv[:], in0=argv[:], in1=t16[:],
                                    op=ALU.add)
            nc.vector.tensor_tensor(out=argv[:], in0=argv[:],
                                    in1=arge[:].to_broadcast([128, TT, 16]),
                                    op=ALU.subtract)
            nc.vector.tensor_tensor(out=argv[:], in0=argv[:],
                                    in1=lnb_b.to_broadcast([128, TT, 16]),
                                    op=ALU.add)
            bern = radp.tile([128, TT, 32], BF16)
            nc.vector.memset(bern[:, :, 16:32], 0.0)
            nc.scalar.activation(bern[:, :, 0:16], argv[:], AF.Exp)

            # global bern transposes -> bernT lhsT layout
            gofs = 0
            for w in range(NW):
                nt = nt_list[w]
                T0 = int(starts[w])
                for gix in range((nt + 3) // 4):
                    gsz = min(4, nt - gix * 4)
                    tp = rps.tile([128, 1024], BF16, space="PSUM", tag="tp",
                                  name="tp")
                    nc.tensor.transpose(
                        out=tp[0:32 * gsz, 0:128],
                        in_=bern[:, T0 + gix * 4: T0 + gix * 4 + gsz, :],
                        identity=identb[:])
                    nc.vector.tensor_copy(bernT[0:32 * gsz, gofs, :],
                                          tp[0:32 * gsz, 0:128])
                    gofs += 1

        # ------------------------------------------------------------------
        # node phase: 3 edge MLPs (idx 1..3), node-major records
        # ------------------------------------------------------------------
        stage_dmas = []
        with (
            tc.tile_pool(name="nod", bufs=3) as nod,
            tc.tile_pool(name="stg", bufs=3) as stg,
            tc.tile_pool(name="nps", bufs=3, space="PSUM") as nps,
            tc.tile_pool(name="tps", bufs=2, space="PSUM") as tps,
        ):
            for ch in range(NCHUNK):
                x_ap = xt_sb[:, ch * 512:(ch + 1) * 512]
                stage_t = stg.tile([128, 4, 384], BF16, tag="stage",
                                   name="stage")
                sw = nod.tile([128, 512], BF16, tag="sw", name="sw")
                nc.scalar.activation(sw[:], x_ap, AF.Silu, scale=sv(1, 0))
                for idx in (1, 2, 3):
                    if idx > 1 and not share_sw:
                        sw = nod.tile([128, 512], BF16, tag="sw", name="sw")
                        nc.scalar.activation(sw[:], x_ap, AF.Silu,
                                             scale=sv(idx, 0))
                    h1 = nps.tile([128, 512], F32, space="PSUM", tag="h",
                                  name="h1")
                    nc.tensor.matmul(out=h1[:], lhsT=wslice(idx, 0), rhs=sw[:],
                                     start=True, stop=True)
                    sw2 = nod.tile([128, 512], BF16, tag="sw2", name="sw2")
                    nc.scalar.activation(sw2[:], h1[:], AF.Silu,
                                         scale=sv(idx, 1), bias=sv(idx, 3))
                    h2 = nps.tile([128, 512], F32, space="PSUM", tag="h",
                                  name="h2")
                    nc.tensor.matmul(out=h2[:], lhsT=wslice(idx, 1), rhs=sw2[:],
                                     start=True, stop=True)
                    r = nod.tile([128, 512], BF16, tag="r", name="r")
                    nc.vector.scalar_tensor_tensor(out=r[:], in0=h2[:],
                                                   scalar=sv(idx, 4),
                                                   in1=x_ap, op0=ALU.add,
                                                   op1=ALU.add)
                    sw3 = nod.tile([128, 512], BF16, tag="sw3", name="sw3")
                    nc.scalar.activation(sw3[:], r[:], AF.Silu, scale=sv(idx, 2))
                    # third layer as swT @ W3 -> node-major [n, f] per block
                    hT = tps.tile([128, 4, 128], F32, space="PSUM", tag="hT",
                                  name="hT")
                    for b in range(4):
                        nc.tensor.matmul(out=hT[:, b, :],
                                         lhsT=sw3[:, b * 128:(b + 1) * 128],
                                         rhs=wslice(idx, 2),
                                         start=True, stop=True)
                    if b3zero:
                        nc.scalar.copy(
                            stage_t[:, :, (idx - 1) * 128: idx * 128], hT[:])
                    else:
                        nc.vector.tensor_tensor(
                            out=stage_t[:, :, (idx - 1) * 128: idx * 128],
                            in0=hT[:],
                            in1=b3rep[:, (idx - 1) * 128: idx * 128]
                            .rearrange("p (o f) -> p o f", o=1)
                            .to_broadcast([128, 4, 128]),
                            op=ALU.add)
                dst = mrec[ch * 512:(ch + 1) * 512, :].rearrange(
                    "(c p) f -> p c f", p=128)
                dma = nc.sync.dma_start(out=dst, in_=stage_t[:])
                stage_dmas.append(dma)
            # cterm (idx 0) on local nodes, feature-major
            for ch in range(LOCPAD // 512):
                x_ap = xtl_sb[:, ch * 512:(ch + 1) * 512]
                sw = nod.tile([128, 512], BF16, tag="sw", name="sw")
                nc.scalar.activation(sw[:], x_ap, AF.Silu, scale=sv(0, 0))
                h1 = nps.tile([128, 512], F32, space="PSUM", tag="h", name="ch1")
                nc.tensor.matmul(out=h1[:], lhsT=wslice(0, 0), rhs=sw[:],
                                 start=True, stop=True)
                sw2 = nod.tile([128, 512], BF16, tag="sw2", name="csw2")
                nc.scalar.activation(sw2[:], h1[:], AF.Silu, scale=sv(0, 1),
                                     bias=sv(0, 3))
                h2 = nps.tile([128, 512], F32, space="PSUM", tag="h", name="ch2")
                nc.tensor.matmul(out=h2[:], lhsT=wslice(0, 1), rhs=sw2[:],
                                 start=True, stop=True)
                r = nod.tile([128, 512], BF16, tag="r", name="cr")
                nc.vector.scalar_tensor_tensor(out=r[:], in0=h2[:],
                                               scalar=sv(0, 4), in1=x_ap,
                                               op0=ALU.add, op1=ALU.add)
                sw3 = nod.tile([128, 512], BF16, tag="sw3", name="csw3")
                nc.scalar.activation(sw3[:], r[:], AF.Silu, scale=sv(0, 2))
                h3 = nps.tile([128, 512], F32, space="PSUM", tag="h", name="ch3")
                nc.tensor.matmul(out=h3[:], lhsT=wslice(0, 2), rhs=sw3[:],
                                 start=True, stop=True)
                nc.vector.tensor_scalar(out=cterm[:, ch * 512:(ch + 1) * 512],
                                        in0=h3[:], scalar1=sv(0, 5),
                                        scalar2=None, op0=ALU.add)

        # ------------------------------------------------------------------
        # edge phase
        # ------------------------------------------------------------------
        with (
            tc.tile_pool(name="rec", bufs=2) as recp,
            tc.tile_pool(name="ohp", bufs=3) as ohp,
            tc.tile_pool(name="ap_", bufs=3) as app,
            tc.tile_pool(name="epi", bufs=3) as epip,
            tc.tile_pool(name="acc_ps", bufs=1, space="PSUM") as accp,
            tc.tile_pool(name="rg_ps", bufs=1, space="PSUM") as rgp,
            tc.tile_pool(name="ep_ps", bufs=1, space="PSUM") as epp,
        ):
            gofs = 0
            gq = 0
            for w in range(NW):
                nt = nt_list[w]
                T0 = int(starts[w])
                ngrp = (nt + 3) // 4

                rec = recp.tile([128, ntmax, 384], BF16, tag="rec", name="rec")
                for g4 in range(ngrp):
                    gsz4 = min(4, nt - g4 * 4)
                    gT = T0 + g4 * 4
                    g = nc.gpsimd.dma_gather(
                        rec[:, g4 * 4:g4 * 4 + gsz4, :], mrec[:],
                        eidx_sb[:, gT * 8:(gT + gsz4) * 8],
                        gsz4 * 128, gsz4 * 128, 384,
                        queue_num=gq)
                    gq = (gq + 1) % 4
                    for sd in stage_dmas:
                        add_dep_helper(g.ins, sd.ins, reason="rec before gather")

                acc = accp.tile([128, 1536], F32, space="PSUM", tag="acc")
                # acc regions: s[0:128] p[128:512] d[512:1024]+[1024:1152]

                for gix in range(ngrp):
                    gsz = min(4, nt - gix * 4)
                    # radG for the 4 tiles of this group (PSUM, 4 banks)
                    rg = rgp.tile([128, 4, 512], F32, space="PSUM", tag="rg",
                                  name="rg")
                    for k in range(gsz):
                        nc.tensor.matmul(
                            out=rg[:, k, 0:384],
                            lhsT=bernT[32 * k:32 * k + 32, gofs, :],
                            rhs=grep_sb[32 * k:32 * k + 32, :],
                            start=True, stop=True,
                            tile_position=(32 * k, 0))
                    # A = rec * radG; radG staged to SBUF bf16 via ACT so the
                    # multiply runs in DVE 2x_1P mode (no PSUM/broadcast src)
                    rgsb = app.tile([128, 4, 384], BF16, tag="rgsb", name="rgsb")
                    nc.scalar.copy(rgsb[:, 0:gsz, :], rg[:, 0:gsz, 0:384])
                    A4 = app.tile([128, 4, 384], BF16, tag="A", name="A")
                    nc.vector.tensor_tensor(
                        out=A4[:, 0:gsz, :],
                        in0=rec[:, gix * 4:gix * 4 + gsz, :],
                        in1=rgsb[:, 0:gsz, :], op=ALU.mult)

                    # batched one-hot construction (bf16); base + d-scales on
                    # GpSimd (broadcast srcs force DVE to 1x mode anyway)
                    ohb = ohp.tile([128, 4, 9, 128], BF16, tag="oh", name="oh")
                    irelb = erele[:, T0 + gix * 4: T0 + gix * 4 + gsz] \
                        .rearrange("p (t o) -> p t o", o=1)
                    nc.vector.tensor_tensor(
                        out=ohb[:, 0:gsz, 0, :],
                        in0=iotae.rearrange("p (o n) -> p o n", o=1)
                        .to_broadcast([128, gsz, 128]),
                        in1=irelb.to_broadcast([128, gsz, 128]),
                        op=ALU.is_equal)
                    ubg = ub[:, T0 + gix * 4:T0 + gix * 4 + gsz, :] \
                        .rearrange("p t (c o) -> p t c o", o=1)
                    nc.vector.tensor_tensor(
                        out=ohb[:, 0:gsz, 1:4, :],
                        in0=ohb[:, 0:gsz, 0:1, :].to_broadcast([128, gsz, 3, 128]),
                        in1=ubg.to_broadcast([128, gsz, 3, 128]),
                        op=ALU.mult)
                    y2g = y2b[:, T0 + gix * 4:T0 + gix * 4 + gsz, :] \
                        .rearrange("p t (c o) -> p t c o", o=1)
                    nc.vector.tensor_tensor(
                        out=ohb[:, 0:gsz, 4:9, :],
                        in0=ohb[:, 0:gsz, 0:1, :].to_broadcast([128, gsz, 5, 128]),
                        in1=y2g.to_broadcast([128, gsz, 5, 128]),
                        op=ALU.mult)

                    for k in range(gsz):
                        t = gix * 4 + k
                        st, sp = (t == 0), (t == nt - 1)
                        A_ = A4[:, k, :]
                        oh_ = ohb[:, k, :, :]
                        nc.tensor.matmul(out=acc[:, 0:128], lhsT=A_[:, 0:128],
                                         rhs=oh_[:, 0, :], start=st, stop=sp)
                        # same PSUM bank as the s region: start=True would
                        # clear the whole bank's has_written, erasing s's t=0
                        # write. One start per bank; rely on overwrite-if-unset.
                        nc.tensor.matmul(out=acc[:, 128:512],
                                         lhsT=A_[:, 128:256],
                                         rhs=oh_[:, 1:4, :], start=False,
                                         stop=sp, skip_group_check=True)
                        nc.tensor.matmul(out=acc[:, 512:1024],
                                         lhsT=A_[:, 256:384],
                                         rhs=oh_[:, 4:8, :], start=st, stop=sp)
                        nc.tensor.matmul(out=acc[:, 1024:1152],
                                         lhsT=A_[:, 256:384],
                                         rhs=oh_[:, 8, :], start=st, stop=sp)
                    gofs += 1

                # ---- window epilogue ----
                qsb = epip.tile([128, 1152], BF16, tag="qsb", name="qsb")
                nc.scalar.copy(qsb[:], acc[:, 0:1152])
                inp = epip.tile([128, 128], F32, tag="inp", name="inp")
                nc.vector.tensor_tensor(out=inp[:], in0=acc[:, 0:128],
                                        in1=cterm[:, w * 128:(w + 1) * 128],
                                        op=ALU.add)
                t1 = epp.tile([128, 512], F32, space="PSUM", tag="scr", name="t1")
                nc.tensor.matmul(out=t1[:, 0:384], lhsT=pdm_sb[:, 0:128],
                                 rhs=qsb[:, 128:512], start=True, stop=True)
                t1sb = epip.tile([128, 512], F32, tag="t1sb", name="t1sb")
                nc.scalar.copy(t1sb[:, 0:384], t1[:, 0:384])
                t2 = epp.tile([128, 512], F32, space="PSUM", tag="scr", name="t2")
                nc.tensor.matmul(out=t2[:, 0:384], lhsT=pdm_sb[:, 128:256],
                                 rhs=qsb[:, 128:512], start=True, stop=True)
                pp = epip.tile([128, 128, 5], F32, tag="pp", name="pp")
                nc.vector.tensor_tensor(
                    out=pp[:, :, 0:3].rearrange("p n c -> p c n"),
                    in0=t1sb[:, 0:384].rearrange("p (c n) -> p c n", n=128),
                    in1=t2[:, 0:384].rearrange("p (c n) -> p c n", n=128),
                    op=ALU.mult)
                red = epip.tile([128, 128], F32, tag="red", name="red")
                nc.vector.tensor_reduce(out=red[:], in_=pp[:, :, 0:3],
                                        axis=mybir.AxisListType.X, op=ALU.add)
                nc.vector.tensor_tensor(out=inp[:], in0=inp[:], in1=red[:],
                                        op=ALU.add)
                t1d = epp.tile([128, 512], F32, space="PSUM", tag="scr", name="t1d")
                nc.tensor.matmul(out=t1d[:, 0:512], lhsT=pdm_sb[:, 256:384],
                                 rhs=qsb[:, 512:1024], start=True, stop=True)
                t1dsb = epip.tile([128, 512], F32, tag="t1sb", name="t1dsb")
                nc.scalar.copy(t1dsb[:], t1d[:])
                t2d = epp.tile([128, 512], F32, space="PSUM", tag="scr", name="t2d")
                nc.tensor.matmul(out=t2d[:, 0:512], lhsT=pdm_sb[:, 384:512],
                                 rhs=qsb[:, 512:1024], start=True, stop=True)
                ppd = epip.tile([128, 128, 5], F32, tag="pp", name="ppd")
                nc.vector.tensor_tensor(
                    out=ppd[:, :, 0:4].rearrange("p n c -> p c n"),
                    in0=t1dsb[:].rearrange("p (c n) -> p c n", n=128),
                    in1=t2d[:].rearrange("p (c n) -> p c n", n=128),
                    op=ALU.mult)
                t1e = epp.tile([128, 512], F32, space="PSUM", tag="scr", name="t1e")
                nc.tensor.matmul(out=t1e[:, 0:128], lhsT=pdm_sb[:, 256:384],
                                 rhs=qsb[:, 1024:1152], start=True, stop=True)
                t1esb = epip.tile([128, 128], F32, tag="t1esb", name="t1esb")
                nc.scalar.copy(t1esb[:], t1e[:, 0:128])
                t2e = epp.tile([128, 512], F32, space="PSUM", tag="scr", name="t2e")
                nc.tensor.matmul(out=t2e[:, 0:128], lhsT=pdm_sb[:, 384:512],
                                 rhs=qsb[:, 1024:1152], start=True, stop=True)
                nc.vector.tensor_tensor(
                    out=ppd[:, :, 4:5].rearrange("p n c -> p c n"),
                    in0=t1esb[:].rearrange("p (c n) -> p c n", n=128),
                    in1=t2e[:, 0:128].rearrange("p (c n) -> p c n", n=128),
                    op=ALU.mult)
                redd = epip.tile([128, 128], F32, tag="red", name="redd")
                nc.vector.tensor_reduce(out=redd[:], in_=ppd[:],
                                        axis=mybir.AxisListType.X, op=ALU.add)
                nc.vector.tensor_tensor(out=inp[:], in0=inp[:], in1=redd[:],
                                        op=ALU.add)
                # final mlp (idx 4)
                sw = epip.tile([128, 128], BF16, tag="fsw", name="fsw")
                nc.scalar.activation(sw[:], inp[:], AF.Silu, scale=sv(4, 0))
                h1 = epp.tile([128, 512], F32, space="PSUM", tag="scr", name="fh1")
                nc.tensor.matmul(out=h1[:, 0:128], lhsT=wslice(4, 0), rhs=sw[:],
                                 start=True, stop=True)
                sw2 = epip.tile([128, 128], BF16, tag="fsw2", name="fsw2")
                nc.scalar.activation(sw2[:], h1[:, 0:128], AF.Silu,
                                     scale=sv(4, 1), bias=sv(4, 3))
                h2 = epp.tile([128, 512], F32, space="PSUM", tag="scr", name="fh2")
                nc.tensor.matmul(out=h2[:, 0:128], lhsT=wslice(4, 1), rhs=sw2[:],
                                 start=True, stop=True)
                r4 = epip.tile([128, 128], BF16, tag="fr", name="fr")
                nc.vector.scalar_tensor_tensor(out=r4[:], in0=h2[:, 0:128],
                                               scalar=sv(4, 4), in1=inp[:],
                                               op0=ALU.add, op1=ALU.add)
                sw3 = epip.tile([128, 128], BF16, tag="fsw3", name="fsw3")
                nc.scalar.activation(sw3[:], r4[:], AF.Silu, scale=sv(4, 2))
                h3 = epp.tile([128, 512], F32, space="PSUM", tag="scr", name="fh3")
                nc.tensor.matmul(out=h3[:, 0:128], lhsT=wslice(4, 2), rhs=sw3[:],
                                 start=True, stop=True)
                outw = epip.tile([128, 128], F32, tag="outw", name="outw")
                nc.vector.tensor_scalar(out=outw[:], in0=h3[:, 0:128],
                                        scalar1=sv(4, 5), scalar2=None,
                                        op0=ALU.add)
                nc.sync.dma_start(out=outT[:, w * 128:(w + 1) * 128],
                                  in_=outw[:])
        es.close()
    nc.compile()
    return nc


# ----------------------------------------------------------------------------
# host side
# ----------------------------------------------------------------------------

def _prep_host(xyz, x_tilde, nbrs, W1, b1, W2, b2, W3, b3, alpha, beta,
               G_s, G_p, G_d, P_1, P_2, D_1, D_2):
    import ml_dtypes
    xyz = np.asarray(xyz, np.float32)
    x_tilde = np.asarray(x_tilde, np.float32)
    nbrs = np.asarray(nbrs)
    i = nbrs[:, 0].astype(np.int64)
    j = nbrs[:, 1].astype(np.int64)
    E = i.shape[0]

    r_ij = (xyz[j] - xyz[i]).astype(np.float32)

    core = i // NPC
    iloc = i - core * NPC
    w = iloc >> 7
    irel = (iloc & 127).astype(np.float32)
    key = core * NW + w
    order = np.argsort(key, kind="stable")
    cnt = np.bincount(key, minlength=N_CORES * NW).reshape(N_CORES, NW)
    n_t = np.maximum(1, -(-cnt.max(axis=0) // 128)).astype(int)
    TT = int(n_t.sum())
    starts = np.concatenate([[0], np.cumsum(n_t)]).astype(int)
    EPAD = TT * 128

    j_pad = np.zeros((N_CORES, EPAD), np.int64)
    irel_pad = np.full((N_CORES, EPAD), 200.0, np.float32)
    rij_pad = np.zeros((N_CORES, EPAD, 3), np.float32)

    cnt_flat = cnt.reshape(-1)
    grp_start = np.concatenate([[0], np.cumsum(cnt_flat)])[:-1]
    pos_in_grp = np.arange(E) - np.repeat(grp_start, cnt_flat)
    core_s = core[order]
    w_s = w[order]
    slot = starts[w_s] * 128 + pos_in_grp
    j_pad[core_s, slot] = j[order]
    irel_pad[core_s, slot] = irel[order]
    rij_pad[core_s, slot] = r_ij[order]

    eidx = np.zeros((N_CORES, 128, TT * 8), np.int16)
    for wi in range(NW):
        nt = int(n_t[wi]); base = int(starts[wi])
        jw = j_pad[:, base * 128:(base + nt) * 128]
        c = np.arange(nt * 8)
        t, q = c // 8, c % 8
        r16 = np.arange(16)
        e_ix = t[None, :] * 128 + r16[:, None] + 16 * q[None, :]
        blk = jw[:, e_ix].astype(np.int16)
        eidx[:, :, base * 8:(base + nt) * 8] = np.tile(blk, (1, 8, 1))

    erel = irel_pad.reshape(N_CORES, TT, 128).transpose(0, 2, 1).copy()
    erij = rij_pad.reshape(N_CORES, TT, 128, 3).transpose(0, 2, 1, 3) \
        .reshape(N_CORES, 128, TT * 3).copy()

    alpha = np.asarray(alpha, np.float64)
    beta = np.asarray(beta, np.float64)
    W1 = np.asarray(W1, np.float64); W2 = np.asarray(W2, np.float64)
    W3 = np.asarray(W3, np.float64)
    b1 = np.asarray(b1, np.float64); b2 = np.asarray(b2, np.float64)
    b3 = np.asarray(b3, np.float64)
    assert np.all(np.abs(beta) > 1e-6), "beta==0 unsupported by silu fold"

    wmats, svcols = [], np.zeros((128, 30), np.float32)
    for idx in range(5):
        a, b = alpha[idx], beta[idx]
        wmats += [(a[0] / b[0])[:, None] * W1[idx],
                  (a[1] / b[1])[:, None] * W2[idx],
                  (a[2] / b[2])[:, None] * W3[idx]]
        svcols[:, idx * 6 + 0] = b[0]
        svcols[:, idx * 6 + 1] = b[1]
        svcols[:, idx * 6 + 2] = b[2]
        svcols[:, idx * 6 + 3] = b[1] * b1[idx]
        svcols[:, idx * 6 + 4] = b2[idx]
        svcols[:, idx * 6 + 5] = b3[idx]
    wst = np.stack(wmats).astype(np.float32).transpose(1, 0, 2) \
        .reshape(128, 15 * 128).copy()

    # fp32 consts
    cf32 = np.zeros((128, 78), np.float32)
    cf32[:, 0:30] = svcols
    ks = np.arange(16, dtype=np.float64)
    cf32[:, 30:46] = (-GAMMA * ks)[None, :]
    cf32[:, 46:62] = (15.0 - ks)[None, :]
    cf32[:, 62:78] = np.log(_BINOM)[None, :]

    grep_np = np.zeros((128, 384), np.float32)
    for q in range(4):
        for X, G in enumerate([G_s, G_p, G_d]):
            grep_np[32 * q:32 * q + 16, X * 128:(X + 1) * 128] = \
                np.asarray(G, np.float32).T
    pdm_np = np.concatenate([np.asarray(M, np.float32).T for M in
                             (P_1, P_2, D_1, D_2)], axis=1)

    # b3 for idx 1..3 replicated across partitions [128, 384]
    b3rep = np.zeros((128, 384), np.float32)
    for idx in (1, 2, 3):
        b3rep[:, (idx - 1) * 128: idx * 128] = b3[idx][None, :]

    xtT_np = np.zeros((128, NPAD), np.float32)
    xtT_np[:, :N_NODES] = x_tilde.T
    xtl_np = np.zeros((N_CORES, 128, LOCPAD), np.float32)
    for cix in range(N_CORES):
        xtl_np[cix, :, :NPC] = x_tilde[cix * NPC:(cix + 1) * NPC].T

    iota_np = np.arange(128, dtype=np.float32)[None, :].repeat(128, 0)
    in_maps = []
    xtT_b = xtT_np.astype(ml_dtypes.bfloat16)
    for cix in range(N_CORES):
        cstb = np.concatenate(
            [iota_np, erel[cix], wst, grep_np, pdm_np, xtl_np[cix], b3rep],
            axis=1).astype(ml_dtypes.bfloat16)
        in_maps.append({
            "xtT": xtT_b, "cstd": cf32, "cstb": cstb,
            "eidx": eidx[cix], "erij": erij[cix],
        })
    share_sw = bool(np.allclose(beta[1, 0], beta[2, 0]) and
                    np.allclose(beta[2, 0], beta[3, 0]))
    b3zero = bool(np.all(b3[1:4] == 0.0))
    return tuple(int(x) for x in n_t), in_maps, share_sw, b3zero


def kernel(**inputs) -> np.ndarray:
    global LAST_EXEC_NS, LAST_RESULT
    n_t, in_maps, share_sw, b3zero = _prep_host(**inputs)
    key = n_t + (share_sw, b3zero)
    if key not in _prog_cache:
        _prog_cache[key] = _build_program(n_t, share_sw=share_sw,
                                          b3zero=b3zero)
    nc = _prog_cache[key]

    trace = os.environ.get("KBENCH_TRACE", "0") == "1"
    res = run_bass_kernel_spmd(nc, in_maps, core_ids=list(range(N_CORES)),
                               trace=trace)
    if trace:
        LAST_EXEC_NS = res.exec_time_ns
        LAST_RESULT = res

    out = np.empty((N_NODES, FEAT), np.float32)
    for cix in range(N_CORES):
        out[cix * NPC:(cix + 1) * NPC] = res.results[cix]["outT"][:, :NPC].T
    return out
